# revision 19
# baseline (speedup 1.0000x reference)
"""Trainium2 Bass kernel for nn_BoundaryLoss (8-core data-parallel).

Math (see reference): loss = (1/C) * sum_c mean_{b,h,w} |pred_sdf_c - tgt_sdf_c|.

For any pred whose global logit spread is < 15, softmax probabilities are
never exactly 0.0 or 1.0 in f32, so both EDTs on the pred side saturate at
theta and pred_sdf == 0 identically (and no pred class-plane is empty).  The
host verifies that bound (np.max - np.min < 15 is sufficient) and falls back
to an exact slow path otherwise.  The device therefore only computes the
target-side SDFs:

  per image b, class c:
    d_plus_c  = min(5, dist to {target==c})      (interior pixels only)
    d_minus_c = min(5, dist to {target!=c}) = min_{c'!=c} d_plus_c'
    |tgt_sdf| = (d_plus + d_minus)/5             (one of the two is 0)

  Exact capped EDT on device (per 512x512 class mask, cap 25 = theta^2):
    The capped EDT is a min-plus convolution with the cone |o|^2, computed
    exactly in exp space: exp(-beta*|o|^2) is a separable 2D Gaussian, so
      S = Wg (x)_y ( Wg (x)_x m )        (two banded-matmul passes on PE,
                                          with a DMA transpose between)
      D2 = round(-ln(S)/beta + (B2+B3)/beta + 0.45), capped at 25
    is exact: D2 candidates are integers, the LSE overshoot is in [0, ~0.43]
    (<= ln(ring)/beta), so with the +0.45 bias both trunc and
    round-to-nearest casts land on the true integer.
    d = sqrt(D2) with fused per-partition row sums (ACT accum_out).
    (Outputs live in x-major layout after the transpose; sums don't care.)

Per core: 2 images (batch shard).  Output: per-partition partial sums of
d_plus / d_minus per (image, class, y-block); host reduces, handles empty
classes, scales, and averages.
"""

import ml_dtypes
import numpy as np

import concourse.bacc as bacc
import concourse.bass as bass
import concourse.mybir as mybir
from concourse.mybir import AluOpType as Op
from concourse.tile import TileContext

P = 128
H = W = 512
YB = H // P          # 4 y-blocks
C = 4                # classes
BPC = 2              # images per core
NCORES = 8
B_TOTAL = BPC * NCORES

BETA = 5.0
B2 = 20.0            # exp bias, conv-Y weights
B3 = 20.0            # exp bias, conv-X weights
RND = 0.35           # rounding bias for the RNE cast; window covers LSE overshoot
                     # [0, 0.4423] plus bf16 u/q rounding (+-0.096): delta in (-0.5, 0.45)

F32 = mybir.dt.float32
BF16 = mybir.dt.bfloat16
I32 = mybir.dt.int32
I8 = mybir.dt.int8
Act = mybir.ActivationFunctionType


def _build_nc():
    nc = bacc.Bacc("TRN2", target_bir_lowering=False, debug=False)
    tgt_d = nc.dram_tensor("target", [BPC, H, W], I32, kind="ExternalInput")
    wm_d = nc.dram_tensor("wmats", [P, 3 * P + YB * W], BF16, kind="ExternalInput")
    osp_d = nc.dram_tensor("osum_p", [P, BPC * YB], F32, kind="ExternalOutput")
    osm_d = nc.dram_tensor("osum_m", [P, BPC * YB], F32, kind="ExternalOutput")

    with TileContext(nc) as tc:
        with (
            tc.tile_pool(name="const", bufs=1) as cpool,
            tc.tile_pool(name="tgt", bufs=3) as tgt_pool,
            tc.tile_pool(name="mask", bufs=YB + 1) as m_pool,
            tc.tile_pool(name="syp", bufs=YB + 1) as sy_pool,
            tc.tile_pool(name="stp", bufs=YB + 1) as st_pool,
            tc.tile_pool(name="lnq", bufs=3) as u_pool,
            tc.tile_pool(name="d2r", bufs=3) as d2_pool,
            tc.tile_pool(name="dmap", bufs=YB + 1) as d_pool,
            tc.tile_pool(name="scratch", bufs=2) as s_pool,
            tc.tile_pool(name="acc", bufs=1) as a_pool,
            tc.tile_pool(name="psy", bufs=4, space="PSUM") as psy_pool,
            tc.tile_pool(name="psx", bufs=4, space="PSUM") as psx_pool,
        ):
            # ---- constants: banded conv weights, host-precomputed ----
            bias0 = cpool.tile([P, 1], F32)
            nc.vector.memset(bias0, 0.0)
            wtile = cpool.tile([P, 3 * P + YB * W], BF16)
            nc.sync.dma_start(wtile, wm_d[:, :])
            wmain = wtile[:, 0:P]
            wprev = wtile[:, P:2 * P]
            wnext = wtile[:, 2 * P:3 * P]
            wrow = [wtile[:, 3 * P + j * W:3 * P + (j + 1) * W] for j in range(YB)]

            accp = a_pool.tile([P, BPC * YB], F32)
            accm = a_pool.tile([P, BPC * YB], F32)

            for b in range(BPC):
                # ---- masks: (t == c) -> {0, 1} bf16 ----
                m_tiles = []
                for yb in range(YB):
                    tgt = tgt_pool.tile([P, W], I32)
                    nc.sync.dma_start(tgt, tgt_d[b, yb * P:(yb + 1) * P, :])
                    tgtb = tgt_pool.tile([P, W], BF16, tag="tgtb")
                    nc.vector.tensor_copy(tgtb, tgt)
                    mw = m_pool.tile([P, C, W], BF16)
                    for c in range(C):
                        nc.vector.tensor_scalar(mw[:, c], tgtb, c, None, Op.is_equal)
                    m_tiles.append(mw)

                # ---- conv-Y fused with transpose on PE:
                #      SyT[x, y] = sum_y' m[y', x] * Wrow[y', y]
                #      (mask block as the stationary operand) ----
                st_tiles = []
                for xb in range(YB):
                    st = st_pool.tile([P, C, W], BF16)
                    for c in range(C):
                        ps = psy_pool.tile([P, W], F32)
                        for j in range(YB):
                            nc.tensor.matmul(
                                ps,
                                m_tiles[j][:, c, xb * P:(xb + 1) * P],
                                wrow[j],
                                start=(j == 0), stop=(j == YB - 1),
                            )
                        if c >= 2:
                            nc.scalar.activation(st[:, c], ps, Act.Copy)
                        else:
                            nc.vector.tensor_copy(st[:, c], ps)
                    st_tiles.append(st)

                # ---- conv-X on PE (in transposed layout) + Ln ----
                u_tiles = []
                for xb in range(YB):
                    uu = u_pool.tile([P, C, W], BF16)
                    ops = [(wmain, xb)]
                    if xb > 0:
                        ops.append((wprev, xb - 1))
                    if xb < YB - 1:
                        ops.append((wnext, xb + 1))
                    for c in range(C):
                        ps = psx_pool.tile([P, W], F32, tag="psx")
                        for i, (wm, src_) in enumerate(ops):
                            nc.tensor.matmul(
                                ps, wm, st_tiles[src_][:, c],
                                start=(i == 0), stop=(i == len(ops) - 1),
                            )
                        nc.scalar.activation(uu[:, c], ps, Act.Ln, bias=bias0[:])
                    u_tiles.append(uu)

                # ---- q = -u/beta + bias ; round-cast to int8 with cap ----
                d2_tiles = []
                for xb in range(YB):
                    qq = u_pool.tile([P, C, W], BF16, tag="qq")
                    nc.vector.tensor_scalar(qq[:], u_tiles[xb][:], -1.0 / BETA,
                                            (B2 + B3) / BETA + RND,
                                            Op.mult, Op.add)
                    d2r = d2_pool.tile([P, C, W], I8)
                    nc.vector.tensor_scalar(d2r[:], qq[:], 25.49, None, Op.min)
                    d2_tiles.append(d2r)

                # ---- d = sqrt(D2), one wide op per block with fused d_plus
                #      row sums; d_minus min-tree interleaved per block ----
                for xb in range(YB):
                    dd = d_pool.tile([P, C, W], BF16)
                    colp = b * YB + xb
                    nc.scalar.activation(
                        dd[:], d2_tiles[xb][:], Act.Sqrt,
                        bias=bias0[:], accum_out=accp[:, colp:colp + 1],
                    )
                    # pair mins in one wide op: mm = [min(d0,d1), min(d2,d3)]
                    mm = s_pool.tile([P, 2, W], BF16, tag="mm")
                    da = dd[:]
                    ap_even = bass.AP(da.tensor, da.offset,
                                      [da.ap[0], [2 * W, 2], [1, W]])
                    ap_odd = bass.AP(da.tensor, da.offset + W,
                                     [da.ap[0], [2 * W, 2], [1, W]])
                    nc.vector.tensor_tensor(mm[:], ap_even, ap_odd, Op.min)
                    # d_minus for all 4 classes in one fused min+row-sum:
                    # in0 = dd in order [d1, d0, d3, d2]; in1 = [m23, m23, m01, m01]
                    ma = mm[:]
                    dm = s_pool.tile([P, C, W], BF16, tag="dm")
                    for g in range(2):
                        # g=0: [d1, d0] vs m23 ; g=1: [d3, d2] vs m01
                        in0 = bass.AP(da.tensor, da.offset + W + g * 2 * W,
                                      [da.ap[0], [-W, 2], [1, W]])
                        in1 = bass.AP(ma.tensor, ma.offset + (1 - g) * W,
                                      [ma.ap[0], [0, 2], [1, W]])
                        nc.vector.tensor_tensor(
                            dm[:, 2 * g:2 * g + 2], in0, in1, Op.min)
                    colm = b * YB + xb
                    dms = s_pool.tile([P, C, W], BF16, tag="dms")
                    nc.vector.tensor_scalar(
                        dms[:], dm[:], 0.0, 0.0, Op.add, Op.add,
                        accum_out=accm[:, colm:colm + 1],
                    )

            nc.sync.dma_start(osp_d[:, :], accp[:])
            nc.sync.dma_start(osm_d[:, :], accm[:])

    nc.compile()
    return nc


_NC = None
_WM = None


def _host_wmats():
    """Banded Gaussian weights, bf16, matching the device formula."""
    global _WM
    if _WM is None:
        k = np.arange(P)[:, None].astype(np.float32)
        cols = []
        specs = [(0, P), (-P, P), (P, P)] + [(j * P, W) for j in range(YB)]
        for base, width in specs:
            m = np.arange(width)[None, :].astype(np.float32)
            idx = base + k - m
            cols.append(np.exp(np.float32(B2) - np.float32(BETA) * idx * idx,
                               dtype=np.float32))
        _WM = np.concatenate(cols, axis=1).astype(ml_dtypes.bfloat16)
    return _WM


def _get_nc():
    global _NC
    if _NC is None:
        _NC = _build_nc()
    return _NC


def _exact_fallback(pred, target):
    """Exact numpy implementation of the reference (slow; adversarial inputs only)."""
    THETA0, THETA, R = 3.0, 5.0, 5
    offs = [(dy, dx, float(np.hypot(dy, dx)))
            for dy in range(-R, R + 1) for dx in range(-R, R + 1)
            if np.hypot(dy, dx) <= THETA]

    def capped_edt(ts):
        B, Hh, Ww = ts.shape
        pad = np.zeros((B, Hh + 2 * R, Ww + 2 * R), bool)
        pad[:, R:-R, R:-R] = ts
        d = np.full((B, Hh, Ww), THETA, np.float32)
        for dy, dx, dist in offs:
            win = pad[:, R + dy:R + dy + Hh, R + dx:R + dx + Ww]
            d = np.minimum(d, np.where(win, np.float32(dist), np.float32(THETA)))
        return d

    def compute_sdf(mask):
        sdf_pos = capped_edt(mask == 1.0)
        sdf_neg = capped_edt(mask == 0.0)
        sdf = np.clip(sdf_pos - sdf_neg, -THETA, THETA) / THETA
        empty = mask.sum(axis=(1, 2)) == 0.0
        return np.where(empty[:, None, None], np.float32(THETA0), sdf).astype(np.float32)

    x = pred.astype(np.float32)
    x = x - x.max(axis=1, keepdims=True)
    ex = np.exp(x)
    p = ex / ex.sum(axis=1, keepdims=True)
    Cn = pred.shape[1]
    loss = np.float32(0.0)
    for c in range(Cn):
        ps = compute_sdf(p[:, c].astype(np.float32))
        ts = compute_sdf((target == c).astype(np.float32))
        loss += np.abs(ps - ts).mean(dtype=np.float32)
    return np.float32(loss / Cn)


def kernel(pred: np.ndarray, target: np.ndarray) -> np.ndarray:
    pred = np.asarray(pred)
    target = np.asarray(target)

    # Soundness guards for the pred_sdf == 0 shortcut and the no-empty-class
    # assumption (neither trips on randn / randint inputs; exact otherwise).
    gap_ok = float(pred.max()) - float(pred.min()) < 15.0
    tgt_ok = bool(((target >= 0) & (target < C)).all())
    present = np.array([[(target[b] == c).any() for c in range(C)]
                        for b in range(B_TOTAL)])
    if not (gap_ok and tgt_ok and present.all()):
        return _exact_fallback(pred, target)

    from concourse.bass_utils import run_bass_kernel_spmd

    nc = _get_nc()
    wm = _host_wmats()
    in_maps = [
        {"target": np.ascontiguousarray(target[i * BPC:(i + 1) * BPC]),
         "wmats": wm}
        for i in range(NCORES)
    ]
    res = run_bass_kernel_spmd(nc, in_maps, list(range(NCORES))).results

    # host reduction: loss = sum(d_plus + d_minus) / (5 * npx * B * C)
    npx = H * W
    total = 0.0
    for core in range(NCORES):
        total += float(res[core]["osum_p"].astype(np.float64).sum())
        total += float(res[core]["osum_m"].astype(np.float64).sum())
    loss = total / (5.0 * npx * B_TOTAL * C)
    return np.float32(loss)


# revision 20
# speedup vs baseline: 1.1104x; 1.1104x over previous
"""Trainium2 Bass kernel for nn_BoundaryLoss (8-core data-parallel).

Math (see reference): loss = (1/C) * sum_c mean_{b,h,w} |pred_sdf_c - tgt_sdf_c|.

For any pred whose global logit spread is < 15, softmax probabilities are
never exactly 0.0 or 1.0 in f32, so both EDTs on the pred side saturate at
theta and pred_sdf == 0 identically (and no pred class-plane is empty).  The
host verifies that bound (np.max - np.min < 15 is sufficient) and falls back
to an exact slow path otherwise.  The device therefore only computes the
target-side SDFs:

  per image b, class c:
    d_plus_c  = min(5, dist to {target==c})      (interior pixels only)
    d_minus_c = min(5, dist to {target!=c}) = min_{c'!=c} d_plus_c'
    |tgt_sdf| = (d_plus + d_minus)/5             (one of the two is 0)

  Exact capped EDT on device (per 512x512 class mask, cap 25 = theta^2):
    The capped EDT is a min-plus convolution with the cone |o|^2, computed
    exactly in exp space: exp(-beta*|o|^2) is a separable 2D Gaussian, so
      S = Wg (x)_y ( Wg (x)_x m )        (two banded-matmul passes on PE,
                                          with a DMA transpose between)
      D2 = round(-ln(S)/beta + (B2+B3)/beta + 0.45), capped at 25
    is exact: D2 candidates are integers, the LSE overshoot is in [0, ~0.43]
    (<= ln(ring)/beta), so with the +0.45 bias both trunc and
    round-to-nearest casts land on the true integer.
    d = sqrt(D2) with fused per-partition row sums (ACT accum_out).
    (Outputs live in x-major layout after the transpose; sums don't care.)

Per core: 2 images (batch shard).  Output: per-partition partial sums of
d_plus / d_minus per (image, class, y-block); host reduces, handles empty
classes, scales, and averages.
"""

import ml_dtypes
import numpy as np

import concourse.bacc as bacc
import concourse.bass as bass
import concourse.mybir as mybir
from concourse.mybir import AluOpType as Op
from concourse.tile import TileContext

P = 128
H = W = 512
YB = H // P          # 4 y-blocks
C = 4                # classes
BPC = 2              # images per core
NCORES = 8
B_TOTAL = BPC * NCORES

BETA = 5.0
B2 = 20.0            # exp bias, conv-Y weights
B3 = 20.0            # exp bias, conv-X weights
RND = 0.35           # rounding bias for the RNE cast; window covers LSE overshoot
                     # [0, 0.4423] plus bf16 u/q rounding (+-0.096): delta in (-0.5, 0.45)

F32 = mybir.dt.float32
BF16 = mybir.dt.bfloat16
I32 = mybir.dt.int32
I8 = mybir.dt.int8
Act = mybir.ActivationFunctionType


def _build_nc():
    nc = bacc.Bacc("TRN2", target_bir_lowering=False, debug=False)
    tgt_d = nc.dram_tensor("target", [BPC, H, W], I32, kind="ExternalInput")
    wm_d = nc.dram_tensor("wmats", [P, 3 * P + YB * W], BF16, kind="ExternalInput")
    osp_d = nc.dram_tensor("osum_p", [P, BPC * YB], F32, kind="ExternalOutput")
    osm_d = nc.dram_tensor("osum_m", [P, BPC * YB * 2], F32, kind="ExternalOutput")

    with TileContext(nc) as tc:
        with (
            tc.tile_pool(name="const", bufs=1) as cpool,
            tc.tile_pool(name="tgt", bufs=3) as tgt_pool,
            tc.tile_pool(name="mask", bufs=YB + 1) as m_pool,
            tc.tile_pool(name="syp", bufs=YB + 1) as sy_pool,
            tc.tile_pool(name="stp", bufs=YB + 1) as st_pool,
            tc.tile_pool(name="lnq", bufs=3) as u_pool,
            tc.tile_pool(name="d2r", bufs=3) as d2_pool,
            tc.tile_pool(name="dmap", bufs=YB + 1) as d_pool,
            tc.tile_pool(name="scratch", bufs=2) as s_pool,
            tc.tile_pool(name="acc", bufs=1) as a_pool,
            tc.tile_pool(name="psy", bufs=4, space="PSUM") as psy_pool,
            tc.tile_pool(name="psx", bufs=4, space="PSUM") as psx_pool,
        ):
            # ---- constants: banded conv weights, host-precomputed ----
            bias0 = cpool.tile([P, 1], F32)
            nc.vector.memset(bias0, 0.0)
            wtile = cpool.tile([P, 3 * P + YB * W], BF16)
            nc.sync.dma_start(wtile, wm_d[:, :])
            wmain = wtile[:, 0:P]
            wprev = wtile[:, P:2 * P]
            wnext = wtile[:, 2 * P:3 * P]
            wrow = [wtile[:, 3 * P + j * W:3 * P + (j + 1) * W] for j in range(YB)]

            accp = a_pool.tile([P, BPC * YB], F32)
            accm = a_pool.tile([P, BPC * YB * 2], F32)

            for b in range(BPC):
                # ---- masks: (t == c) -> {0, 1} bf16 ----
                m_tiles = []
                for yb in range(YB):
                    tgt = tgt_pool.tile([P, W], I32)
                    nc.sync.dma_start(tgt, tgt_d[b, yb * P:(yb + 1) * P, :])
                    tgtb = tgt_pool.tile([P, W], BF16, tag="tgtb")
                    nc.vector.tensor_copy(tgtb, tgt)
                    mw = m_pool.tile([P, C, W], BF16)
                    for c in range(C):
                        nc.vector.tensor_scalar(mw[:, c], tgtb, c, None, Op.is_equal)
                    m_tiles.append(mw)

                # ---- conv-Y fused with transpose on PE:
                #      SyT[x, y] = sum_y' m[y', x] * Wrow[y', y]
                #      (mask block as the stationary operand) ----
                st_tiles = []
                for xb in range(YB):
                    st = st_pool.tile([P, C, W], BF16)
                    for c in range(C):
                        ps = psy_pool.tile([P, W], F32)
                        for j in range(YB):
                            nc.tensor.matmul(
                                ps,
                                m_tiles[j][:, c, xb * P:(xb + 1) * P],
                                wrow[j],
                                start=(j == 0), stop=(j == YB - 1),
                            )
                        if c >= 2:
                            nc.scalar.activation(st[:, c], ps, Act.Copy)
                        else:
                            nc.vector.tensor_copy(st[:, c], ps)
                    st_tiles.append(st)

                # ---- conv-X on PE (in transposed layout) + Ln ----
                u_tiles = []
                for xb in range(YB):
                    uu = u_pool.tile([P, C, W], BF16)
                    ops = [(wmain, xb)]
                    if xb > 0:
                        ops.append((wprev, xb - 1))
                    if xb < YB - 1:
                        ops.append((wnext, xb + 1))
                    for c in range(C):
                        ps = psx_pool.tile([P, W], F32, tag="psx")
                        for i, (wm, src_) in enumerate(ops):
                            nc.tensor.matmul(
                                ps, wm, st_tiles[src_][:, c],
                                start=(i == 0), stop=(i == len(ops) - 1),
                            )
                        nc.scalar.activation(uu[:, c], ps, Act.Ln, bias=bias0[:])
                    u_tiles.append(uu)

                # ---- q = -u/beta + bias ; round-cast to int8 with cap ----
                d2_tiles = []
                for xb in range(YB):
                    qq = u_pool.tile([P, C, W], BF16, tag="qq")
                    nc.vector.tensor_scalar(qq[:], u_tiles[xb][:], -1.0 / BETA,
                                            (B2 + B3) / BETA + RND,
                                            Op.mult, Op.add)
                    d2r = d2_pool.tile([P, C, W], I8)
                    nc.vector.tensor_scalar(d2r[:], qq[:], 25.49, None, Op.min)
                    d2_tiles.append(d2r)

                # ---- d = sqrt(D2), one wide op per block with fused d_plus
                #      row sums; d_minus min-tree interleaved per block ----
                for xb in range(YB):
                    dd = d_pool.tile([P, C, W], BF16)
                    colp = b * YB + xb
                    nc.scalar.activation(
                        dd[:], d2_tiles[xb][:], Act.Sqrt,
                        bias=bias0[:], accum_out=accp[:, colp:colp + 1],
                    )
                    # pair mins in one wide op: mm = [min(d0,d1), min(d2,d3)]
                    mm = s_pool.tile([P, 2, W], BF16, tag="mm")
                    da = dd[:]
                    ap_even = bass.AP(da.tensor, da.offset,
                                      [da.ap[0], [2 * W, 2], [1, W]])
                    ap_odd = bass.AP(da.tensor, da.offset + W,
                                     [da.ap[0], [2 * W, 2], [1, W]])
                    nc.vector.tensor_tensor(mm[:], ap_even, ap_odd, Op.min)
                    # d_minus for all 4 classes in one fused min+row-sum:
                    # in0 = dd in order [d1, d0, d3, d2]; in1 = [m23, m23, m01, m01]
                    ma = mm[:]
                    dm = s_pool.tile([P, C, W], BF16, tag="dm")
                    for g in range(2):
                        # g=0: [d1, d0] vs m23 ; g=1: [d3, d2] vs m01
                        in0 = bass.AP(da.tensor, da.offset + W + g * 2 * W,
                                      [da.ap[0], [-W, 2], [1, W]])
                        in1 = bass.AP(ma.tensor, ma.offset + (1 - g) * W,
                                      [ma.ap[0], [0, 2], [1, W]])
                        colm = (b * YB + xb) * 2 + g
                        nc.vector.scalar_tensor_tensor(
                            dm[:, 2 * g:2 * g + 2], in0, 0.0, in1,
                            Op.add, Op.min,
                            accum_out=accm[:, colm:colm + 1],
                        )

            nc.sync.dma_start(osp_d[:, :], accp[:])
            nc.sync.dma_start(osm_d[:, :], accm[:])

    nc.compile()
    return nc


_NC = None
_WM = None


def _host_wmats():
    """Banded Gaussian weights, bf16, matching the device formula."""
    global _WM
    if _WM is None:
        k = np.arange(P)[:, None].astype(np.float32)
        cols = []
        specs = [(0, P), (-P, P), (P, P)] + [(j * P, W) for j in range(YB)]
        for base, width in specs:
            m = np.arange(width)[None, :].astype(np.float32)
            idx = base + k - m
            cols.append(np.exp(np.float32(B2) - np.float32(BETA) * idx * idx,
                               dtype=np.float32))
        _WM = np.concatenate(cols, axis=1).astype(ml_dtypes.bfloat16)
    return _WM


def _get_nc():
    global _NC
    if _NC is None:
        _NC = _build_nc()
    return _NC


def _exact_fallback(pred, target):
    """Exact numpy implementation of the reference (slow; adversarial inputs only)."""
    THETA0, THETA, R = 3.0, 5.0, 5
    offs = [(dy, dx, float(np.hypot(dy, dx)))
            for dy in range(-R, R + 1) for dx in range(-R, R + 1)
            if np.hypot(dy, dx) <= THETA]

    def capped_edt(ts):
        B, Hh, Ww = ts.shape
        pad = np.zeros((B, Hh + 2 * R, Ww + 2 * R), bool)
        pad[:, R:-R, R:-R] = ts
        d = np.full((B, Hh, Ww), THETA, np.float32)
        for dy, dx, dist in offs:
            win = pad[:, R + dy:R + dy + Hh, R + dx:R + dx + Ww]
            d = np.minimum(d, np.where(win, np.float32(dist), np.float32(THETA)))
        return d

    def compute_sdf(mask):
        sdf_pos = capped_edt(mask == 1.0)
        sdf_neg = capped_edt(mask == 0.0)
        sdf = np.clip(sdf_pos - sdf_neg, -THETA, THETA) / THETA
        empty = mask.sum(axis=(1, 2)) == 0.0
        return np.where(empty[:, None, None], np.float32(THETA0), sdf).astype(np.float32)

    x = pred.astype(np.float32)
    x = x - x.max(axis=1, keepdims=True)
    ex = np.exp(x)
    p = ex / ex.sum(axis=1, keepdims=True)
    Cn = pred.shape[1]
    loss = np.float32(0.0)
    for c in range(Cn):
        ps = compute_sdf(p[:, c].astype(np.float32))
        ts = compute_sdf((target == c).astype(np.float32))
        loss += np.abs(ps - ts).mean(dtype=np.float32)
    return np.float32(loss / Cn)


def kernel(pred: np.ndarray, target: np.ndarray) -> np.ndarray:
    pred = np.asarray(pred)
    target = np.asarray(target)

    # Soundness guards for the pred_sdf == 0 shortcut and the no-empty-class
    # assumption (neither trips on randn / randint inputs; exact otherwise).
    gap_ok = float(pred.max()) - float(pred.min()) < 15.0
    tgt_ok = bool(((target >= 0) & (target < C)).all())
    present = np.array([[(target[b] == c).any() for c in range(C)]
                        for b in range(B_TOTAL)])
    if not (gap_ok and tgt_ok and present.all()):
        return _exact_fallback(pred, target)

    from concourse.bass_utils import run_bass_kernel_spmd

    nc = _get_nc()
    wm = _host_wmats()
    in_maps = [
        {"target": np.ascontiguousarray(target[i * BPC:(i + 1) * BPC]),
         "wmats": wm}
        for i in range(NCORES)
    ]
    res = run_bass_kernel_spmd(nc, in_maps, list(range(NCORES))).results

    # host reduction: loss = sum(d_plus + d_minus) / (5 * npx * B * C)
    npx = H * W
    total = 0.0
    for core in range(NCORES):
        total += float(res[core]["osum_p"].astype(np.float64).sum())
        total += float(res[core]["osum_m"].astype(np.float64).sum())
    loss = total / (5.0 * npx * B_TOTAL * C)
    return np.float32(loss)


# revision 21
# speedup vs baseline: 1.1417x; 1.0282x over previous
"""Trainium2 Bass kernel for nn_BoundaryLoss (8-core data-parallel).

Math (see reference): loss = (1/C) * sum_c mean_{b,h,w} |pred_sdf_c - tgt_sdf_c|.

For any pred whose global logit spread is < 15, softmax probabilities are
never exactly 0.0 or 1.0 in f32, so both EDTs on the pred side saturate at
theta and pred_sdf == 0 identically (and no pred class-plane is empty).  The
host verifies that bound (np.max - np.min < 15 is sufficient) and falls back
to an exact slow path otherwise.  The device therefore only computes the
target-side SDFs:

  per image b, class c:
    d_plus_c  = min(5, dist to {target==c})      (interior pixels only)
    d_minus_c = min(5, dist to {target!=c}) = min_{c'!=c} d_plus_c'
    |tgt_sdf| = (d_plus + d_minus)/5             (one of the two is 0)

  Exact capped EDT on device (per 512x512 class mask, cap 25 = theta^2):
    The capped EDT is a min-plus convolution with the cone |o|^2, computed
    exactly in exp space: exp(-beta*|o|^2) is a separable 2D Gaussian, so
      S = Wg (x)_y ( Wg (x)_x m )        (two banded-matmul passes on PE,
                                          with a DMA transpose between)
      D2 = round(-ln(S)/beta + (B2+B3)/beta + 0.45), capped at 25
    is exact: D2 candidates are integers, the LSE overshoot is in [0, ~0.43]
    (<= ln(ring)/beta), so with the +0.45 bias both trunc and
    round-to-nearest casts land on the true integer.
    d = sqrt(D2) with fused per-partition row sums (ACT accum_out).
    (Outputs live in x-major layout after the transpose; sums don't care.)

Per core: 2 images (batch shard).  Output: per-partition partial sums of
d_plus / d_minus per (image, class, y-block); host reduces, handles empty
classes, scales, and averages.
"""

import ml_dtypes
import numpy as np

import concourse.bacc as bacc
import concourse.bass as bass
import concourse.mybir as mybir
from concourse.mybir import AluOpType as Op
from concourse.tile import TileContext

P = 128
H = W = 512
YB = H // P          # 4 y-blocks
C = 4                # classes
BPC = 2              # images per core
NCORES = 8
B_TOTAL = BPC * NCORES

BETA = 5.0
B2 = 20.0            # exp bias, conv-Y weights
B3 = 20.0            # exp bias, conv-X weights
RND = 0.35           # rounding bias for the RNE cast; window covers LSE overshoot
                     # [0, 0.4423] plus bf16 u/q rounding (+-0.096): delta in (-0.5, 0.45)

F32 = mybir.dt.float32
BF16 = mybir.dt.bfloat16
I32 = mybir.dt.int32
I8 = mybir.dt.int8
Act = mybir.ActivationFunctionType


def _build_nc():
    nc = bacc.Bacc("TRN2", target_bir_lowering=False, debug=False)
    tgt_d = nc.dram_tensor("target", [BPC, H, W], I32, kind="ExternalInput")
    wm_d = nc.dram_tensor("wmats", [P, 3 * P + YB * W], BF16, kind="ExternalInput")
    osp_d = nc.dram_tensor("osum_p", [P, BPC * YB], F32, kind="ExternalOutput")
    osm_d = nc.dram_tensor("osum_m", [P, BPC * YB * 2], F32, kind="ExternalOutput")

    with TileContext(nc) as tc:
        with (
            tc.tile_pool(name="const", bufs=1) as cpool,
            tc.tile_pool(name="tgt", bufs=3) as tgt_pool,
            tc.tile_pool(name="mask", bufs=YB + 2) as m_pool,
            tc.tile_pool(name="syp", bufs=YB + 1) as sy_pool,
            tc.tile_pool(name="stp", bufs=YB + 2) as st_pool,
            tc.tile_pool(name="lnq", bufs=5) as u_pool,
            tc.tile_pool(name="d2r", bufs=5) as d2_pool,
            tc.tile_pool(name="dmap", bufs=YB + 1) as d_pool,
            tc.tile_pool(name="scratch", bufs=3) as s_pool,
            tc.tile_pool(name="acc", bufs=1) as a_pool,
            tc.tile_pool(name="psy", bufs=4, space="PSUM") as psy_pool,
            tc.tile_pool(name="psx", bufs=4, space="PSUM") as psx_pool,
        ):
            # ---- constants: banded conv weights, host-precomputed ----
            bias0 = cpool.tile([P, 1], F32)
            nc.vector.memset(bias0, 0.0)
            wtile = cpool.tile([P, 3 * P + YB * W], BF16)
            nc.sync.dma_start(wtile, wm_d[:, :])
            wmain = wtile[:, 0:P]
            wprev = wtile[:, P:2 * P]
            wnext = wtile[:, 2 * P:3 * P]
            wrow = [wtile[:, 3 * P + j * W:3 * P + (j + 1) * W] for j in range(YB)]

            accp = a_pool.tile([P, BPC * YB], F32)
            accm = a_pool.tile([P, BPC * YB * 2], F32)

            for b in range(BPC):
                # ---- masks: (t == c) -> {0, 1} bf16 ----
                m_tiles = []
                for yb in range(YB):
                    tgt = tgt_pool.tile([P, W], I32)
                    nc.sync.dma_start(tgt, tgt_d[b, yb * P:(yb + 1) * P, :])
                    tgtb = tgt_pool.tile([P, W], BF16, tag="tgtb")
                    nc.vector.tensor_copy(tgtb, tgt)
                    mw = m_pool.tile([P, C, W], BF16)
                    for c in range(C):
                        nc.vector.tensor_scalar(mw[:, c], tgtb, c, None, Op.is_equal)
                    m_tiles.append(mw)

                # ---- conv-Y fused with transpose on PE:
                #      SyT[x, y] = sum_y' m[y', x] * Wrow[y', y]
                #      (mask block as the stationary operand) ----
                st_tiles = []
                for xb in range(YB):
                    st = st_pool.tile([P, C, W], BF16)
                    for c in range(C):
                        ps = psy_pool.tile([P, W], F32)
                        for j in range(YB):
                            nc.tensor.matmul(
                                ps,
                                m_tiles[j][:, c, xb * P:(xb + 1) * P],
                                wrow[j],
                                start=(j == 0), stop=(j == YB - 1),
                            )
                        if c >= 2:
                            nc.scalar.activation(st[:, c], ps, Act.Copy)
                        else:
                            nc.vector.tensor_copy(st[:, c], ps)
                    st_tiles.append(st)

                # ---- conv-X on PE (in transposed layout) + Ln ----
                u_tiles = []
                for xb in range(YB):
                    uu = u_pool.tile([P, C, W], BF16)
                    ops = [(wmain, xb)]
                    if xb > 0:
                        ops.append((wprev, xb - 1))
                    if xb < YB - 1:
                        ops.append((wnext, xb + 1))
                    for c in range(C):
                        ps = psx_pool.tile([P, W], F32, tag="psx")
                        for i, (wm, src_) in enumerate(ops):
                            nc.tensor.matmul(
                                ps, wm, st_tiles[src_][:, c],
                                start=(i == 0), stop=(i == len(ops) - 1),
                            )
                        nc.scalar.activation(uu[:, c], ps, Act.Ln, bias=bias0[:])
                    u_tiles.append(uu)

                # ---- q = -u/beta + bias ; round-cast to int8 with cap ----
                d2_tiles = []
                for xb in range(YB):
                    qq = u_pool.tile([P, C, W], BF16, tag="qq")
                    nc.vector.tensor_scalar(qq[:], u_tiles[xb][:], -1.0 / BETA,
                                            (B2 + B3) / BETA + RND,
                                            Op.mult, Op.add)
                    d2r = d2_pool.tile([P, C, W], I8)
                    nc.vector.tensor_scalar(d2r[:], qq[:], 25.49, None, Op.min)
                    d2_tiles.append(d2r)

                # ---- d = sqrt(D2), one wide op per block with fused d_plus
                #      row sums; d_minus min-tree interleaved per block ----
                for xb in range(YB):
                    dd = d_pool.tile([P, C, W], BF16)
                    colp = b * YB + xb
                    nc.scalar.activation(
                        dd[:], d2_tiles[xb][:], Act.Sqrt,
                        bias=bias0[:], accum_out=accp[:, colp:colp + 1],
                    )
                    # pair mins in one wide op: mm = [min(d0,d1), min(d2,d3)]
                    mm = s_pool.tile([P, 2, W], BF16, tag="mm")
                    da = dd[:]
                    ap_even = bass.AP(da.tensor, da.offset,
                                      [da.ap[0], [2 * W, 2], [1, W]])
                    ap_odd = bass.AP(da.tensor, da.offset + W,
                                     [da.ap[0], [2 * W, 2], [1, W]])
                    nc.vector.tensor_tensor(mm[:], ap_even, ap_odd, Op.min)
                    # d_minus for all 4 classes in one fused min+row-sum:
                    # in0 = dd in order [d1, d0, d3, d2]; in1 = [m23, m23, m01, m01]
                    ma = mm[:]
                    dm = s_pool.tile([P, C, W], BF16, tag="dm")
                    for g in range(2):
                        # g=0: [d1, d0] vs m23 ; g=1: [d3, d2] vs m01
                        in0 = bass.AP(da.tensor, da.offset + W + g * 2 * W,
                                      [da.ap[0], [-W, 2], [1, W]])
                        in1 = bass.AP(ma.tensor, ma.offset + (1 - g) * W,
                                      [ma.ap[0], [0, 2], [1, W]])
                        colm = (b * YB + xb) * 2 + g
                        nc.vector.scalar_tensor_tensor(
                            dm[:, 2 * g:2 * g + 2], in0, 0.0, in1,
                            Op.add, Op.min,
                            accum_out=accm[:, colm:colm + 1],
                        )

            nc.sync.dma_start(osp_d[:, :], accp[:])
            nc.sync.dma_start(osm_d[:, :], accm[:])

    nc.compile()
    return nc


_NC = None
_WM = None


def _host_wmats():
    """Banded Gaussian weights, bf16, matching the device formula."""
    global _WM
    if _WM is None:
        k = np.arange(P)[:, None].astype(np.float32)
        cols = []
        specs = [(0, P), (-P, P), (P, P)] + [(j * P, W) for j in range(YB)]
        for base, width in specs:
            m = np.arange(width)[None, :].astype(np.float32)
            idx = base + k - m
            cols.append(np.exp(np.float32(B2) - np.float32(BETA) * idx * idx,
                               dtype=np.float32))
        _WM = np.concatenate(cols, axis=1).astype(ml_dtypes.bfloat16)
    return _WM


def _get_nc():
    global _NC
    if _NC is None:
        _NC = _build_nc()
    return _NC


def _exact_fallback(pred, target):
    """Exact numpy implementation of the reference (slow; adversarial inputs only)."""
    THETA0, THETA, R = 3.0, 5.0, 5
    offs = [(dy, dx, float(np.hypot(dy, dx)))
            for dy in range(-R, R + 1) for dx in range(-R, R + 1)
            if np.hypot(dy, dx) <= THETA]

    def capped_edt(ts):
        B, Hh, Ww = ts.shape
        pad = np.zeros((B, Hh + 2 * R, Ww + 2 * R), bool)
        pad[:, R:-R, R:-R] = ts
        d = np.full((B, Hh, Ww), THETA, np.float32)
        for dy, dx, dist in offs:
            win = pad[:, R + dy:R + dy + Hh, R + dx:R + dx + Ww]
            d = np.minimum(d, np.where(win, np.float32(dist), np.float32(THETA)))
        return d

    def compute_sdf(mask):
        sdf_pos = capped_edt(mask == 1.0)
        sdf_neg = capped_edt(mask == 0.0)
        sdf = np.clip(sdf_pos - sdf_neg, -THETA, THETA) / THETA
        empty = mask.sum(axis=(1, 2)) == 0.0
        return np.where(empty[:, None, None], np.float32(THETA0), sdf).astype(np.float32)

    x = pred.astype(np.float32)
    x = x - x.max(axis=1, keepdims=True)
    ex = np.exp(x)
    p = ex / ex.sum(axis=1, keepdims=True)
    Cn = pred.shape[1]
    loss = np.float32(0.0)
    for c in range(Cn):
        ps = compute_sdf(p[:, c].astype(np.float32))
        ts = compute_sdf((target == c).astype(np.float32))
        loss += np.abs(ps - ts).mean(dtype=np.float32)
    return np.float32(loss / Cn)


def kernel(pred: np.ndarray, target: np.ndarray) -> np.ndarray:
    pred = np.asarray(pred)
    target = np.asarray(target)

    # Soundness guards for the pred_sdf == 0 shortcut and the no-empty-class
    # assumption (neither trips on randn / randint inputs; exact otherwise).
    gap_ok = float(pred.max()) - float(pred.min()) < 15.0
    tgt_ok = bool(((target >= 0) & (target < C)).all())
    present = np.array([[(target[b] == c).any() for c in range(C)]
                        for b in range(B_TOTAL)])
    if not (gap_ok and tgt_ok and present.all()):
        return _exact_fallback(pred, target)

    from concourse.bass_utils import run_bass_kernel_spmd

    nc = _get_nc()
    wm = _host_wmats()
    in_maps = [
        {"target": np.ascontiguousarray(target[i * BPC:(i + 1) * BPC]),
         "wmats": wm}
        for i in range(NCORES)
    ]
    res = run_bass_kernel_spmd(nc, in_maps, list(range(NCORES))).results

    # host reduction: loss = sum(d_plus + d_minus) / (5 * npx * B * C)
    npx = H * W
    total = 0.0
    for core in range(NCORES):
        total += float(res[core]["osum_p"].astype(np.float64).sum())
        total += float(res[core]["osum_m"].astype(np.float64).sum())
    loss = total / (5.0 * npx * B_TOTAL * C)
    return np.float32(loss)
